# revision 4
# baseline (speedup 1.0000x reference)
"""CCConvLayer (GNN message passing) on 8 Trainium2 NeuronCores.

Reference:
    x1  = x @ W.T                      # dense projection [N, 128]
    out = relu(segment_sum(x1[src] * vals[:, None], dst, N))

v2 strategy — the projection commutes with the (linear) segment-sum:
    out = relu(segment_sum(x[src] * vals[:, None], dst, N) @ W.T)
so there is NO phase-1 x1 table at all.  Per core:

  * dst space is cut into 128-node blocks; blocks are assigned to
    (core, slot) pairs balanced by edge count.  Every core owns the full
    output rows of its blocks => no collective; the host re-assembles.
  * Edges gather raw x rows (bf16, 256B) straight from DRAM with
    dma_gather (int16 indices => lo/hi table halves, sorted by src for
    HBM locality).
  * Scatter-add per 128-edge tile: one-hot S[e, n] = vals[e] *
    (dst_local[e] == n); the tile matmul uses the GATHERED tile as
    lhsT:  psum[ci, n] += G[e, ci].T-contraction S[e, n], which
    accumulates the TRANSPOSED block sum, so the final W application is
    a single plain matmul per slot:
        out[n, co] = relu( accT[ci, n].T @ W.T[ci, co] )  (lhsT=accT!)
  * S tiles are built on TWO engines to halve the critical path:
    DVE scalar_tensor_tensor (is_equal * val) and Pool local_scatter
    (8-tile batches, idx = dst_local + 128*batch_pos, -1 pads skipped).
  * psum->SBUF copies and the final ReLU run on the Activation engine,
    which is otherwise idle.
"""

import math

import numpy as np
import ml_dtypes

import concourse.bacc as bacc
import concourse.bass as bass
import concourse.mybir as mybir
import concourse.tile as tile
from concourse.bass_utils import run_bass_kernel_spmd

P = 128          # partitions / block size / edge-tile size
CH = 128         # in/out channels (problem-specific)
N_CORES = 8
H_SPLIT = 32768  # int16 index limit for dma_gather
GC = 48          # gather chunk size in tiles (6144 edges / call)
XPAD = 512       # x row padding granularity
POOL_BLK = 24    # of every 24 consecutive tiles ...
POOL_N = 8       # ... the first 8 are built on Pool (local_scatter batch)

F32 = mybir.dt.float32
BF16 = mybir.dt.bfloat16
I32 = mybir.dt.int32
I16 = mybir.dt.int16


def _wrap_idx(idx):
    """int16 index layout for dma_gather: element i at partition i%16,
    column i//16; 16-partition block replicated to all 128 partitions."""
    L = len(idx) // 16
    w = idx.reshape(L, 16).T.astype(np.int16)  # [16, L]
    return np.ascontiguousarray(np.tile(w, (8, 1)))  # [128, L]


def _tile_schedule(K):
    """Engine assignment for the K global tiles: list of ('dve', q) and
    ('pool', q0, B) batch entries, in global tile order."""
    sched = []
    q = 0
    while q < K:
        blk = min(POOL_BLK, K - q)
        pb = min(POOL_N, blk)
        if pb >= 2:  # local_scatter needs >= 1 tile; batch of 1 still fine
            sched.append(("pool", q, pb))
        else:
            for j in range(pb):
                sched.append(("dve", q + j))
        for j in range(pb, blk):
            sched.append(("dve", q + j))
        q += blk
    return sched


def _pool_pos(K):
    """pos[q] = batch position j if tile q is pool-built else -1."""
    pos = np.full(K, -1, dtype=np.int64)
    for ent in _tile_schedule(K):
        if ent[0] == "pool":
            _, q0, B = ent
            pos[q0:q0 + B] = np.arange(B)
    return pos


def _plan_edges(src, dst, vals, n_nodes, n_cores, h_split):
    """Bucket edges by 128-node dst block, assign blocks to (slot, core),
    split each slot's edges into lo (src < h_split) / hi sections, pad each
    (slot, core, section) to T*128 edges shared across cores."""
    nb = math.ceil(n_nodes / P)
    nb_pad = math.ceil(nb / n_cores) * n_cores
    slots = nb_pad // n_cores

    blk = (dst // P).astype(np.int64)
    counts = np.bincount(blk, minlength=nb_pad)
    order = np.argsort(-counts, kind="stable")

    assign = np.empty((slots, n_cores), dtype=np.int64)
    totals = np.zeros(n_cores, dtype=np.int64)
    for s in range(slots):
        group = order[s * n_cores:(s + 1) * n_cores]  # desc counts
        cs = np.argsort(totals, kind="stable")  # least-loaded cores first
        for i, b in enumerate(group):
            assign[s, cs[i]] = b
            totals[cs[i]] += counts[b]

    # per-edge-per-core grouping
    eorder = np.argsort(blk, kind="stable")
    starts = np.zeros(nb_pad + 1, dtype=np.int64)
    np.cumsum(counts, out=starts[1:])

    # edge lists per (slot, core, section)
    lists = [[None] * n_cores for _ in range(slots)]
    TL = np.zeros(slots, dtype=np.int64)
    TH = np.zeros(slots, dtype=np.int64)
    for s in range(slots):
        for c in range(n_cores):
            b = int(assign[s, c])
            e = eorder[starts[b]:starts[b + 1]]
            lo = e[src[e] < h_split]
            hi = e[src[e] >= h_split]
            # sort by src: descriptors then walk the x table monotonically,
            # turning random 256B HBM reads into row-local ones
            lo = lo[np.argsort(src[lo], kind="stable")]
            hi = hi[np.argsort(src[hi], kind="stable")]
            lists[s][c] = (lo, hi)
            TL[s] = max(TL[s], -(-len(lo) // P))
            TH[s] = max(TH[s], -(-len(hi) // P))
        if TL[s] + TH[s] == 0:
            TL[s] = 1  # keep the psum chain non-empty
    KL = int(TL.sum())
    KH = int(TH.sum())
    K = KL + KH
    offL = np.zeros(slots + 1, dtype=np.int64)
    np.cumsum(TL, out=offL[1:])
    offH = np.zeros(slots + 1, dtype=np.int64)
    np.cumsum(TH, out=offH[1:])

    # global tile order: per slot, lo tiles then hi tiles (matches emission)
    # guv[u] = global order position of meta column u (u = lo: offL[s]+t,
    # hi: KL + offH[s]+t)
    guv = np.zeros(K, dtype=np.int64)
    g = 0
    for s in range(slots):
        for t in range(TL[s]):
            guv[offL[s] + t] = g
            g += 1
        for t in range(TH[s]):
            guv[KL + offH[s] + t] = g
            g += 1
    ppos = _pool_pos(K)  # indexed by global order position

    srcL = np.zeros((n_cores, KL * P), dtype=np.int64)
    srcH = np.zeros((n_cores, KH * P), dtype=np.int64)
    dstl_a = np.zeros((n_cores, K * P), dtype=np.float32)
    val_a = np.zeros((n_cores, K * P), dtype=np.float32)
    for s in range(slots):
        for c in range(n_cores):
            b = int(assign[s, c])
            lo, hi = lists[s][c]
            ll = int(offL[s]) * P
            srcL[c, ll:ll + len(lo)] = src[lo]
            dstl_a[c, ll:ll + len(lo)] = (dst[lo] - b * P).astype(np.float32)
            val_a[c, ll:ll + len(lo)] = vals[lo]
            ho = int(offH[s]) * P
            srcH[c, ho:ho + len(hi)] = src[hi] - h_split
            hh = (KL + int(offH[s])) * P
            dstl_a[c, hh:hh + len(hi)] = (dst[hi] - b * P).astype(np.float32)
            val_a[c, hh:hh + len(hi)] = vals[hi]

    idxL = np.stack([_wrap_idx(srcL[c]) for c in range(n_cores)]) \
        if KL else np.zeros((n_cores, P, 0), dtype=np.int16)
    idxH = np.stack([_wrap_idx(srcH[c]) for c in range(n_cores)]) \
        if KH else np.zeros((n_cores, P, 0), dtype=np.int16)

    # interleave dstl/vals: position j -> (tile j//P, partition j%P) => [P, K]
    dstl_i = dstl_a.reshape(n_cores, K, P).transpose(0, 2, 1)  # [C, P, K]
    val_i = val_a.reshape(n_cores, K, P).transpose(0, 2, 1)
    # re-index S metadata columns into GLOBAL EMISSION order (per slot:
    # lo tiles then hi tiles), so pool batches slice contiguous columns
    inv = np.argsort(guv)  # inv[g] = meta col u at global position g
    dstl_g = dstl_i[:, :, inv]
    val_g = val_i[:, :, inv]

    # meta_f: [dl (K) | vl2 (2K)] bf16; meta_i: dli (2K) int16, cols by g.
    # vl2 col 2g = vals, col 2g+1 = 0;  dli col 2g = dst_local + 128*batchpos
    # (pool tiles, else unused), col 2g+1 = -1.
    vl2 = np.zeros((n_cores, P, 2 * K), dtype=np.float32)
    vl2[:, :, 0::2] = val_g
    dli = np.full((n_cores, P, 2 * K), -1, dtype=np.int16)
    off = np.where(ppos >= 0, ppos * P, 0)  # [K] col offset within batch
    dli[:, :, 0::2] = (dstl_g + off[None, None, :]).astype(np.int16)
    meta_f = np.ascontiguousarray(
        np.concatenate([dstl_g, vl2], axis=2).astype(ml_dtypes.bfloat16))
    meta_i = np.ascontiguousarray(dli)

    plan = {
        "assign": assign,
        "h": h_split,
        "slots": slots,
        "TL": TL.tolist(),
        "TH": TH.tolist(),
        "KL": KL,
        "KH": KH,
        "offL": offL.tolist(),
        "offH": offH.tolist(),
    }
    return plan, idxL, idxH, meta_f, meta_i


def _build_nc(xrows, plan, n_cores, loop_n=1, mode="full"):
    """Build the SPMD Bass program (identical on every core).

    loop_n > 1 wraps the body in an on-device repeat loop; mode
    ("full" | "gonly" | "nog" | "nos") ablates phases for timing."""
    nc = bacc.Bacc(
        "TRN2",
        target_bir_lowering=False,
        debug=False,
        enable_asserts=False,
        num_devices=n_cores,
        num_swdge_queues=4,
    )
    KL, KH = plan["KL"], plan["KH"]
    K = KL + KH
    slots = plan["slots"]
    # raw x rows, node-major bf16 [xrows, CH] — the gather table
    xt_d = nc.dram_tensor("xtab", [xrows, CH], BF16, kind="ExternalInput").ap()
    wt_d = nc.dram_tensor("wt", [CH, CH], BF16, kind="ExternalInput").ap()
    mf_d = nc.dram_tensor("metaf", [P, 3 * K], BF16, kind="ExternalInput").ap()
    mi_d = nc.dram_tensor("metai", [P, 2 * K], I16, kind="ExternalInput").ap()
    il_d = (
        nc.dram_tensor("idxlo", [P, KL * 8], I16, kind="ExternalInput").ap()
        if KL else None
    )
    ih_d = (
        nc.dram_tensor("idxhi", [P, KH * 8], I16, kind="ExternalInput").ap()
        if KH else None
    )
    out_d = nc.dram_tensor("out", [slots * P, CH], F32, kind="ExternalOutput").ap()

    with tile.TileContext(nc) as tc:
        if loop_n > 1:
            with tc.For_i(0, loop_n, 1):
                _emit_body(nc, tc, plan, xrows, xt_d, wt_d, mf_d, mi_d,
                           il_d, ih_d, out_d, mode)
        else:
            _emit_body(nc, tc, plan, xrows, xt_d, wt_d, mf_d, mi_d,
                       il_d, ih_d, out_d, mode)
    nc.compile()
    return nc


def _emit_body(nc, tc, plan, xrows, xt_d, wt_d, mf_d, mi_d, il_d, ih_d,
               out_d, mode="full"):
    slots = plan["slots"]
    TL, TH = plan["TL"], plan["TH"]
    KL, KH = plan["KL"], plan["KH"]
    offL, offH = plan["offL"], plan["offH"]
    K = KL + KH
    ppos = _pool_pos(K)

    with (
        tc.tile_pool(name="const", bufs=1) as constp,
        tc.tile_pool(name="gat", bufs=8) as gp,
        tc.tile_pool(name="sel", bufs=8) as selp,
        tc.tile_pool(name="sel8", bufs=3) as sel8p,
        tc.tile_pool(name="acc", bufs=3) as accp,
        tc.tile_pool(name="res", bufs=3) as resp,
        tc.tile_pool(name="ps", bufs=6, space="PSUM") as psp,
        tc.tile_pool(name="psf", bufs=2, space="PSUM") as psfp,
    ):
        wt_sb = constp.tile([CH, CH], BF16)
        nc.sync.dma_start(out=wt_sb[:], in_=wt_d[:])
        mf_sb = constp.tile([P, 3 * K], BF16)
        nc.sync.dma_start(out=mf_sb[:], in_=mf_d[:])
        dl_sb = mf_sb[:, :K]
        vl2_sb = mf_sb[:, K:]
        mi_sb = constp.tile([P, 2 * K], I16)
        nc.sync.dma_start(out=mi_sb[:], in_=mi_d[:])
        if KL:
            il_sb = constp.tile([P, KL * 8], I16)
            nc.sync.dma_start(out=il_sb[:], in_=il_d[:])
        if KH:
            ih_sb = constp.tile([P, KH * 8], I16)
            nc.sync.dma_start(out=ih_sb[:], in_=ih_d[:])
        iota_i = constp.tile([P, P], I32)
        nc.gpsimd.iota(iota_i[:], pattern=[[1, P]], base=0, channel_multiplier=0)
        iota_f = constp.tile([P, P], BF16)
        nc.vector.tensor_copy(iota_f[:], iota_i[:])

        # ---- bulk gather: chunked dma_gather per section, lazy issue ----
        chunks = {}  # (sec, chunk_id) -> (tile, tiles_in_chunk)
        qrr = [0]

        def chunk_of(sec, t):
            cid = t // GC
            key = (sec, cid)
            if key not in chunks:
                ksec = KL if sec == 0 else KH
                nt = min(GC, ksec - cid * GC)
                g = gp.tile([P, nt * CH], BF16, tag="gat")
                isb = il_sb if sec == 0 else ih_sb
                h = min(plan["h"], xrows)
                table = xt_d[:h, :] if sec == 0 else xt_d[h:, :]
                nc.gpsimd.dma_gather(
                    out_ap=g[:].rearrange("p (t c) -> p t c", c=CH),
                    in_ap=table,
                    idxs_ap=isb[:, cid * GC * 8:(cid * GC + nt) * 8],
                    num_idxs=nt * P,
                    num_idxs_reg=nt * P,
                    elem_size=CH,
                    single_packet=False,
                    queue_num=qrr[0],
                )
                qrr[0] = (qrr[0] + 1) % 4
                if mode == "gonly":
                    dummy = selp.tile([P, 1], F32, tag="dmy")
                    nc.vector.tensor_copy(dummy[:], g[:, :1])
                chunks[key] = (g, nt)
            return chunks[key]

        # ---- S tiles: pool-batched local_scatter for pool-assigned runs ----
        # built lazily per batch, keyed by global tile position of batch head
        s8tiles = {}

        def pool_s8(q0, B):
            if q0 not in s8tiles:
                S8 = sel8p.tile([P, POOL_N * P], BF16, tag="s8")
                nc.gpsimd.local_scatter(
                    out_ap=S8[:, :B * P],
                    data_ap=vl2_sb[:, 2 * q0:2 * (q0 + B)],
                    idxs_ap=mi_sb[:, 2 * q0:2 * (q0 + B)],
                    channels=P, num_elems=B * P, num_idxs=2 * B,
                )
                s8tiles[q0] = S8
            return s8tiles[q0]

        # schedule entries indexed by global position -> engine + batch info
        K_sched = {}
        for ent in _tile_schedule(K):
            if ent[0] == "pool":
                _, q0, B = ent
                for j in range(B):
                    K_sched[q0 + j] = ("pool", q0, B, j)
            else:
                K_sched[ent[1]] = ("dve",)

        # ---- per-slot psum chains ----
        g = 0  # global tile position (emission order)
        for s in range(slots):
            tiles = [(0, offL[s] + t) for t in range(TL[s])]
            tiles += [(1, offH[s] + t) for t in range(TH[s])]
            if mode == "gonly":
                for sec, t in tiles:
                    chunk_of(sec, t)
                g += len(tiles)
                continue
            ps = psp.tile([CH, P], F32, tag="ps")
            for i, (sec, t) in enumerate(tiles):
                gi = g + i  # global tile position == S-meta column
                if mode == "nog":
                    gt = wt_sb[:]
                else:
                    gtile, _ = chunk_of(sec, t)
                    gt = gtile[:, (t % GC) * CH:(t % GC + 1) * CH]
                if mode == "nos":
                    S_ap = iota_f[:]
                else:
                    ent = K_sched[gi]
                    if ent[0] == "pool":
                        _, q0, B, j = ent
                        S8 = pool_s8(q0, B)
                        S_ap = S8[:, j * P:(j + 1) * P]
                    else:
                        S = selp.tile([P, P], BF16, tag="sel")
                        nc.vector.scalar_tensor_tensor(
                            out=S[:],
                            in0=iota_f[:],
                            scalar=dl_sb[:, gi:gi + 1],
                            in1=vl2_sb[:, 2 * gi:2 * gi + 1].to_broadcast([P, P]),
                            op0=mybir.AluOpType.is_equal,
                            op1=mybir.AluOpType.mult,
                        )
                        S_ap = S[:]
                nc.tensor.matmul(
                    out=ps[:],
                    lhsT=gt,
                    rhs=S_ap,
                    start=(i == 0),
                    stop=(i == len(tiles) - 1),
                )
            g += len(tiles)
            # accT[ci, n] -> bf16 SBUF (Act engine), then out = accT.T @ W.T
            accT = accp.tile([CH, P], BF16, tag="acc")
            nc.scalar.copy(accT[:], ps[:])
            psf = psfp.tile([P, CH], F32, tag="psf")
            nc.tensor.matmul(
                out=psf[:], lhsT=accT[:], rhs=wt_sb[:], start=True, stop=True,
            )
            res = resp.tile([P, CH], F32, tag="res")
            nc.scalar.activation(
                out=res[:], in_=psf[:], func=mybir.ActivationFunctionType.Relu
            )
            nc.sync.dma_start(out=out_d[s * P:(s + 1) * P, :], in_=res[:])


_NC_CACHE = {}


def prepare(x, W, src, dst, vals, n_cores=N_CORES, h_split=H_SPLIT):
    """Host-side planning + input maps."""
    x = np.asarray(x, dtype=np.float32)
    W = np.asarray(W, dtype=np.float32)
    src = np.asarray(src).astype(np.int64)
    dst = np.asarray(dst).astype(np.int64)
    vals = np.asarray(vals, dtype=np.float32)

    n = x.shape[0]
    plan, idxL, idxH, meta_f, meta_i = _plan_edges(
        src, dst, vals, n, n_cores, h_split)

    xrows = math.ceil(n / XPAD) * XPAD
    x_pad = np.zeros((xrows, CH), dtype=ml_dtypes.bfloat16)
    x_pad[:n] = x.astype(ml_dtypes.bfloat16)
    wt = np.ascontiguousarray(W.T).astype(ml_dtypes.bfloat16)

    key = (xrows, n_cores, plan["h"], plan["KL"], plan["KH"],
           tuple(plan["TL"]), tuple(plan["TH"]))
    nc = _NC_CACHE.get(key)
    if nc is None:
        nc = _build_nc(xrows, plan, n_cores)
        _NC_CACHE[key] = nc

    in_maps = []
    for c in range(n_cores):
        m = {"xtab": x_pad, "wt": wt, "metaf": meta_f[c], "metai": meta_i[c]}
        if plan["KL"]:
            m["idxlo"] = idxL[c]
        if plan["KH"]:
            m["idxhi"] = idxH[c]
        in_maps.append(m)
    return nc, in_maps, plan, n


def assemble(results, plan, n, n_cores=N_CORES):
    """Scatter per-core slot outputs back to the full [n, CH] output."""
    assign, slots = plan["assign"], plan["slots"]
    out_full = np.zeros((slots * n_cores * P, CH), dtype=np.float32)
    for c in range(n_cores):
        o = results[c]["out"]
        for s in range(slots):
            b = int(assign[s, c])
            out_full[b * P:(b + 1) * P] = o[s * P:(s + 1) * P]
    return out_full[:n]


def kernel(x, W, src, dst, vals, **_run_kwargs):
    nc, in_maps, plan, n = prepare(x, W, src, dst, vals)
    res = run_bass_kernel_spmd(
        nc, in_maps, core_ids=list(range(N_CORES)), **_run_kwargs
    )
    out = assemble(res.results, plan, n)
    if _run_kwargs:
        return out, res
    return out


# revision 26
# speedup vs baseline: 3.3152x; 3.3152x over previous
"""CCConvLayer (GNN message passing) on 8 Trainium2 NeuronCores.

Reference:
    x1  = x @ W.T                      # dense projection [N, 128]
    out = relu(segment_sum(x1[src] * vals[:, None], dst, N))

v2 strategy — the projection commutes with the (linear) segment-sum:
    out = relu(segment_sum(x[src] * vals[:, None], dst, N) @ W.T)
so there is NO phase-1 x1 table at all.  Per core:

  * dst space is cut into 128-node blocks; blocks are assigned to
    (core, slot) pairs balanced by edge count.  Every core owns the full
    output rows of its blocks => no collective; the host re-assembles.
  * Edges gather raw x rows (bf16, 256B) straight from DRAM with
    dma_gather (int16 indices => lo/hi table halves, sorted by src for
    HBM locality).
  * Scatter-add per 128-edge tile: one-hot S[e, n] = vals[e] *
    (dst_local[e] == n); the tile matmul uses the GATHERED tile as
    lhsT:  psum[ci, n] += G[e, ci].T-contraction S[e, n], which
    accumulates the TRANSPOSED block sum, so the final W application is
    a single plain matmul per slot:
        out[n, co] = relu( accT[ci, n].T @ W.T[ci, co] )  (lhsT=accT!)
  * S tiles are built on TWO engines to halve the critical path:
    DVE scalar_tensor_tensor (is_equal * val) and Pool local_scatter
    (8-tile batches, idx = dst_local + 128*batch_pos, -1 pads skipped).
  * psum->SBUF copies and the final ReLU run on the Activation engine,
    which is otherwise idle.
"""

import math

import numpy as np
import ml_dtypes

import concourse.bacc as bacc
import concourse.bass as bass
import concourse.mybir as mybir
import concourse.tile as tile
from concourse.bass_utils import run_bass_kernel_spmd

P = 128          # partitions / edge-tile size
BW = 128         # dst-block width (S-matrix columns, psum free dim)
CH = 128         # in/out channels (problem-specific)
N_CORES = 8
H_SPLIT = 20224  # table half-split: each phase's working set fits the DMA cache
                 # (must stay < 32768 for int16 dma_gather indices)
GC = 48          # gather chunk size in tiles (6144 edges / call)
NQ = 4           # SWDGE queues used for gathers
SP = False       # single_packet for dma_gather
GB = 8           # gather pool buffers
IDG = 0          # 1 = per-tile indirect_dma_start gather (int32 offsets)
IDT = 1          # tiles per indirect_dma_start call
NEGPAD = 0       # 1 = pad gather idxs are -1 (broken: crashes DGE); 2 = memset pad tiles only
SORT = 1         # sort each (slot, section)'s edges by src
XPAD = 512       # x row padding granularity
SB = 16          # edge-tiles per wide-S DVE build (2 ops per batch)
ACCB = 88        # accT pool buffers
WS = 0           # 1 = wide-S batched builds; 0 = per-tile eager stt builds
SELB = 110       # S pool buffers

F32 = mybir.dt.float32
BF16 = mybir.dt.bfloat16
I32 = mybir.dt.int32
I16 = mybir.dt.int16


def _wrap_idx(idx):
    """int16 index layout for dma_gather: element i at partition i%16,
    column i//16; 16-partition block replicated to all 128 partitions."""
    L = len(idx) // 16
    w = idx.reshape(L, 16).T.astype(np.int16)  # [16, L]
    return np.ascontiguousarray(np.tile(w, (8, 1)))  # [128, L]


def _plan_edges(src, dst, vals, n_nodes, n_cores, h_split):
    """Bucket edges by 128-node dst block, assign blocks to (slot, core),
    split each slot's edges into lo (src < h_split) / hi sections, pad each
    (slot, core, section) to T*128 edges shared across cores."""
    nb = math.ceil(n_nodes / BW)
    nb_pad = math.ceil(nb / n_cores) * n_cores
    slots = nb_pad // n_cores

    blk = (dst // BW).astype(np.int64)
    counts = np.bincount(blk, minlength=nb_pad)
    order = np.argsort(-counts, kind="stable")

    assign = np.empty((slots, n_cores), dtype=np.int64)
    totals = np.zeros(n_cores, dtype=np.int64)
    for s in range(slots):
        group = order[s * n_cores:(s + 1) * n_cores]  # desc counts
        cs = np.argsort(totals, kind="stable")  # least-loaded cores first
        for i, b in enumerate(group):
            assign[s, cs[i]] = b
            totals[cs[i]] += counts[b]

    # per-edge-per-core grouping
    eorder = np.argsort(blk, kind="stable")
    starts = np.zeros(nb_pad + 1, dtype=np.int64)
    np.cumsum(counts, out=starts[1:])

    # edge lists per (slot, core, section)
    lists = [[None] * n_cores for _ in range(slots)]
    TL = np.zeros(slots, dtype=np.int64)
    TH = np.zeros(slots, dtype=np.int64)
    for s in range(slots):
        for c in range(n_cores):
            b = int(assign[s, c])
            e = eorder[starts[b]:starts[b + 1]]
            lo = e[src[e] < h_split]
            hi = e[src[e] >= h_split]
            # sort by src: descriptors then walk the x table monotonically,
            # turning random 256B HBM reads into row-local ones
            if SORT:
                lo = lo[np.argsort(src[lo], kind="stable")]
                hi = hi[np.argsort(src[hi], kind="stable")]
            lists[s][c] = (lo, hi)
            TL[s] = max(TL[s], -(-len(lo) // P))
            TH[s] = max(TH[s], -(-len(hi) // P))
        if TL[s] + TH[s] == 0:
            TL[s] = 1  # keep the psum chain non-empty
    KL = int(TL.sum())
    KH = int(TH.sum())
    K = KL + KH
    offL = np.zeros(slots + 1, dtype=np.int64)
    np.cumsum(TL, out=offL[1:])
    offH = np.zeros(slots + 1, dtype=np.int64)
    np.cumsum(TH, out=offH[1:])

    # global tile order: per slot, lo tiles then hi tiles (matches emission)
    # guv[u] = global order position of meta column u (u = lo: offL[s]+t,
    # hi: KL + offH[s]+t)
    guv = np.zeros(K, dtype=np.int64)
    g = 0
    for s in range(slots):
        for t in range(TL[s]):
            guv[offL[s] + t] = g
            g += 1
    for s in range(slots):
        for t in range(TH[s]):
            guv[KL + offH[s] + t] = g
            g += 1
    fill = -1 if NEGPAD == 1 else 0
    srcL = np.full((n_cores, KL * P), fill, dtype=np.int64)
    srcH = np.full((n_cores, KH * P), fill, dtype=np.int64)
    dstl_a = np.zeros((n_cores, K * P), dtype=np.float32)
    val_a = np.zeros((n_cores, K * P), dtype=np.float32)
    srca_a = np.zeros((n_cores, K * P), dtype=np.int64)
    for s in range(slots):
        for c in range(n_cores):
            b = int(assign[s, c])
            lo, hi = lists[s][c]
            ll = int(offL[s]) * P
            srcL[c, ll:ll + len(lo)] = src[lo]
            srca_a[c, ll:ll + len(lo)] = src[lo]
            dstl_a[c, ll:ll + len(lo)] = (dst[lo] - b * BW).astype(np.float32)
            val_a[c, ll:ll + len(lo)] = vals[lo]
            ho = int(offH[s]) * P
            srcH[c, ho:ho + len(hi)] = src[hi] - h_split
            hh = (KL + int(offH[s])) * P
            srca_a[c, hh:hh + len(hi)] = src[hi]
            dstl_a[c, hh:hh + len(hi)] = (dst[hi] - b * BW).astype(np.float32)
            val_a[c, hh:hh + len(hi)] = vals[hi]

    idxL = np.stack([_wrap_idx(srcL[c]) for c in range(n_cores)]) \
        if KL else np.zeros((n_cores, P, 0), dtype=np.int16)
    idxH = np.stack([_wrap_idx(srcH[c]) for c in range(n_cores)]) \
        if KH else np.zeros((n_cores, P, 0), dtype=np.int16)

    # interleave dstl/vals: position j -> (tile j//P, partition j%P) => [P, K]
    dstl_i = dstl_a.reshape(n_cores, K, P).transpose(0, 2, 1)  # [C, P, K]
    val_i = val_a.reshape(n_cores, K, P).transpose(0, 2, 1)
    srca_i = srca_a.reshape(n_cores, K, P).transpose(0, 2, 1)
    # re-index S metadata columns into GLOBAL EMISSION order (per slot:
    # lo tiles then hi tiles), so pool batches slice contiguous columns
    inv = np.argsort(guv)  # inv[g] = meta col u at global position g
    dstl_g = dstl_i[:, :, inv]
    val_g = val_i[:, :, inv]
    msrc = np.ascontiguousarray(srca_i[:, :, inv]).astype(np.int32)

    if WS:
        # dl2[p, g] = dst_local + BW * (g % SB): column index within the
        # wide-S batch buffer [P, SB*BW].  f32 (bf16 cannot represent
        # 0..SB*BW-1 exactly)
        off = (np.arange(K) % SB) * BW
        mdl = np.ascontiguousarray(
            (dstl_g + off[None, None, :]).astype(np.float32))
    else:
        # per-tile stt: plain dst_local (< BW <= 128, bf16-exact)
        mdl = np.ascontiguousarray(dstl_g.astype(ml_dtypes.bfloat16))
    mvl = np.ascontiguousarray(val_g.astype(ml_dtypes.bfloat16))

    # tiles (sec, t) that contain pad slots in ANY core (memset before gather)
    padset = set()
    for s_ in range(slots):
        for c in range(n_cores):
            lo, hi = lists[s_][c]
            for ln, T_, off_, sec in ((len(lo), TL[s_], offL[s_], 0),
                                      (len(hi), TH[s_], offH[s_], 1)):
                full = -(-ln // P) if ln else 0
                for t_ in range(full - 1 if ln % P else full, T_):
                    if t_ >= 0:
                        padset.add((sec, int(off_) + t_))
    plan = {
        "assign": assign,
        "h": h_split,
        "slots": slots,
        "TL": TL.tolist(),
        "TH": TH.tolist(),
        "KL": KL,
        "KH": KH,
        "offL": offL.tolist(),
        "offH": offH.tolist(),
        "padset": sorted(padset),
    }
    return plan, idxL, idxH, mdl, mvl, msrc


def _build_nc(xrows, plan, n_cores, loop_n=1, mode="full"):
    """Build the SPMD Bass program (identical on every core).

    loop_n > 1 wraps the body in an on-device repeat loop; mode
    ("full" | "gonly" | "nog" | "nos") ablates phases for timing."""
    nc = bacc.Bacc(
        "TRN2",
        target_bir_lowering=False,
        debug=False,
        enable_asserts=False,
        num_devices=n_cores,
        num_swdge_queues=NQ,
    )
    KL, KH = plan["KL"], plan["KH"]
    K = KL + KH
    slots = plan["slots"]
    # raw x rows, node-major bf16 [xrows, CH] — the gather table
    xt_d = nc.dram_tensor("xtab", [xrows, CH], BF16, kind="ExternalInput").ap()
    wt_d = nc.dram_tensor("wt", [CH, CH], BF16, kind="ExternalInput").ap()
    mf_d = nc.dram_tensor("mdl", [P, K], F32 if WS else BF16,
                          kind="ExternalInput").ap()
    mi_d = nc.dram_tensor("mvl", [P, K], BF16, kind="ExternalInput").ap()
    ms_d = (nc.dram_tensor("msrc", [P, K], I32, kind="ExternalInput").ap()
            if IDG else None)
    il_d = (
        nc.dram_tensor("idxlo", [P, KL * 8], I16, kind="ExternalInput").ap()
        if KL and not IDG else None
    )
    ih_d = (
        nc.dram_tensor("idxhi", [P, KH * 8], I16, kind="ExternalInput").ap()
        if KH and not IDG else None
    )
    out_d = nc.dram_tensor("out", [slots * BW, CH], F32, kind="ExternalOutput").ap()

    with tile.TileContext(nc) as tc:
        if loop_n > 1:
            with tc.For_i(0, loop_n, 1):
                _emit_body(nc, tc, plan, xrows, xt_d, wt_d, mf_d, mi_d,
                           il_d, ih_d, ms_d, out_d, mode)
        else:
            _emit_body(nc, tc, plan, xrows, xt_d, wt_d, mf_d, mi_d,
                       il_d, ih_d, ms_d, out_d, mode)
    nc.compile()
    return nc


def _emit_body(nc, tc, plan, xrows, xt_d, wt_d, mf_d, mi_d, il_d, ih_d,
               ms_d, out_d, mode="full"):
    slots = plan["slots"]
    TL, TH = plan["TL"], plan["TH"]
    KL, KH = plan["KL"], plan["KH"]
    offL, offH = plan["offL"], plan["offH"]
    K = KL + KH

    with (
        tc.tile_pool(name="const", bufs=1) as constp,
        tc.tile_pool(name="gat", bufs=GB) as gp,
        tc.tile_pool(name="sel", bufs=SELB) as selp,
        tc.tile_pool(name="acc", bufs=ACCB) as accp,
        tc.tile_pool(name="res", bufs=3) as resp,
        tc.tile_pool(name="ps", bufs=6, space="PSUM") as psp,
        tc.tile_pool(name="psf", bufs=2, space="PSUM") as psfp,
    ):
        wt_sb = constp.tile([CH, CH], BF16)
        nc.sync.dma_start(out=wt_sb[:], in_=wt_d[:])
        dl_sb = constp.tile([P, K], F32 if WS else BF16)
        nc.sync.dma_start(out=dl_sb[:], in_=mf_d[:])
        vl_sb = constp.tile([P, K], BF16)
        nc.sync.dma_start(out=vl_sb[:], in_=mi_d[:])
        if IDG:
            ms_sb = constp.tile([P, K], I32)
            nc.sync.dma_start(out=ms_sb[:], in_=ms_d[:])
        if KL and not IDG:
            il_sb = constp.tile([P, KL * 8], I16)
            nc.sync.dma_start(out=il_sb[:], in_=il_d[:])
        if KH and not IDG:
            ih_sb = constp.tile([P, KH * 8], I16)
            nc.sync.dma_start(out=ih_sb[:], in_=ih_d[:])
        iw = SB * BW if WS else BW
        iota_i = constp.tile([P, iw], I32)
        nc.gpsimd.iota(iota_i[:], pattern=[[1, iw]], base=0,
                       channel_multiplier=0)
        iota_f = constp.tile([P, iw], F32 if WS else BF16)
        nc.vector.tensor_copy(iota_f[:], iota_i[:])

        # ---- bulk gather: chunked dma_gather per section, lazy issue ----
        chunks = {}  # (sec, chunk_id) -> (tile, tiles_in_chunk)
        qrr = [0]
        padset = set(map(tuple, plan.get("padset", ()))) if NEGPAD else set()

        idg_tiles = {}

        def idg_gather(gi):
            b0 = (gi // IDT) * IDT
            if b0 not in idg_tiles:
                nt = min(IDT, K - b0)
                g = gp.tile([P, nt * CH], BF16, tag="gat")
                nc.gpsimd.indirect_dma_start(
                    out=g[:].rearrange("p (t c) -> p t c", c=CH),
                    out_offset=None,
                    in_=xt_d[:],
                    in_offset=bass.IndirectOffsetOnAxis(
                        ap=ms_sb[:, b0:b0 + nt], axis=0),
                )
                if mode == "gonly":
                    dummy = selp.tile([P, 1], F32, tag="dmy")
                    nc.vector.tensor_copy(dummy[:], g[:, :1])
                idg_tiles[b0] = g
            return idg_tiles[b0][:, ((gi - b0) * CH):((gi - b0 + 1) * CH)]

        def chunk_of(sec, t):
            cid = t // GC
            key = (sec, cid)
            if key not in chunks:
                ksec = KL if sec == 0 else KH
                nt = min(GC, ksec - cid * GC)
                g = gp.tile([P, nt * CH], BF16, tag="gat")
                for tt in range(cid * GC, cid * GC + nt):
                    if (sec, tt) in padset:
                        j = tt - cid * GC
                        nc.any.memset(g[:, j * CH:(j + 1) * CH], 0.0)
                isb = il_sb if sec == 0 else ih_sb
                h = min(plan["h"], xrows)
                table = xt_d[:h, :] if sec == 0 else xt_d[h:, :]
                nc.gpsimd.dma_gather(
                    out_ap=g[:].rearrange("p (t c) -> p t c", c=CH),
                    in_ap=table,
                    idxs_ap=isb[:, cid * GC * 8:(cid * GC + nt) * 8],
                    num_idxs=nt * P,
                    num_idxs_reg=nt * P,
                    elem_size=CH,
                    single_packet=SP,
                    queue_num=qrr[0],
                )
                qrr[0] = (qrr[0] + 1) % NQ
                if not WS:
                    for tt in range(cid * GC, cid * GC + nt):
                        gtt = t2g.get((sec, tt))
                        if gtt is not None:
                            build_s(gtt)

                if mode == "gonly":
                    dummy = selp.tile([P, 1], F32, tag="dmy")
                    nc.vector.tensor_copy(dummy[:], g[:, :1])
                chunks[key] = (g, nt)
            return chunks[key]

        # ---- wide-S: one [P, SB*BW] buffer = SB edge-tiles, 2 DVE ops ----
        wdict = {}
        sdict = {}

        def build_s(gi):
            if gi in sdict or mode in ("nos", "gonly"):
                return
            S = selp.tile([P, BW], BF16, tag="sel")
            nc.vector.scalar_tensor_tensor(
                out=S[:],
                in0=iota_f[:, :BW],
                scalar=dl_sb[:, gi:gi + 1],
                in1=vl_sb[:, gi:gi + 1].to_broadcast([P, BW]),
                op0=mybir.AluOpType.is_equal,
                op1=mybir.AluOpType.mult,
            )
            sdict[gi] = S

        def wide_s(b):
            if b not in wdict:
                nt = min(SB, K - b * SB)
                W_ = selp.tile([P, SB * BW], BF16, tag="selw")
                c0 = b * SB
                nc.vector.tensor_tensor(
                    out=W_[:, :nt * BW],
                    in0=dl_sb[:, c0:c0 + nt].to_broadcast([P, nt, BW]),
                    in1=iota_f[:, :nt * BW].rearrange(
                        "p (t n) -> p t n", n=BW),
                    op=mybir.AluOpType.is_equal,
                )
                nc.vector.tensor_tensor(
                    out=W_[:, :nt * BW],
                    in0=W_[:, :nt * BW],
                    in1=vl_sb[:, c0:c0 + nt].to_broadcast([P, nt, BW]),
                    op=mybir.AluOpType.mult,
                )
                wdict[b] = W_
            return wdict[b]

        # ---- two-phase psum chains: all lo sections, then all hi ----
        acc_lo = {}
        g = 0  # global tile position (emission order)
        t2g = {}
        _gg = 0
        for _s in range(slots):
            for _t in range(TL[_s]):
                t2g[(0, offL[_s] + _t)] = _gg
                _gg += 1
        for _s in range(slots):
            for _t in range(TH[_s]):
                t2g[(1, offH[_s] + _t)] = _gg
                _gg += 1

        def chain(sec, s, T, off):
            nonlocal g
            tiles = [(sec, off + t) for t in range(T)]
            if mode == "gonly":
                for i, (sc, t) in enumerate(tiles):
                    idg_gather(g + i) if IDG else chunk_of(sc, t)
                g += len(tiles)
                return None
            ps = psp.tile([CH, BW], F32, tag="ps")
            for i, (sc, t) in enumerate(tiles):
                gi = g + i  # global tile position == S-meta column
                if mode == "nog":
                    gt = wt_sb[:]
                elif IDG:
                    gt = idg_gather(gi)
                else:
                    gtile, _ = chunk_of(sc, t)
                    gt = gtile[:, (t % GC) * CH:(t % GC + 1) * CH]
                if mode == "nos":
                    S_ap = wt_sb[:, :BW]
                elif WS:
                    W_ = wide_s(gi // SB)
                    j = gi % SB
                    S_ap = W_[:, j * BW:(j + 1) * BW]
                else:
                    if gi not in sdict:
                        build_s(gi)
                    S_ap = sdict.pop(gi)[:]
                nc.tensor.matmul(
                    out=ps[:],
                    lhsT=gt,
                    rhs=S_ap,
                    start=(i == 0),
                    stop=(i == len(tiles) - 1),
                )
            g += len(tiles)
            accT = accp.tile([CH, BW], BF16, tag="acc")
            nc.scalar.copy(accT[:], ps[:])
            return accT

        slot_accs = {s: [] for s in range(slots)}
        for s in range(slots):
            if TL[s]:
                a = chain(0, s, TL[s], offL[s])
                if a is not None:
                    slot_accs[s].append(a)
        for s in range(slots):
            if TH[s]:
                a = chain(1, s, TH[s], offH[s])
                if a is not None:
                    slot_accs[s].append(a)
        # final W application deferred so PE never stalls mid-stream on the
        # chain-end copies
        if mode != "gonly":
            for s in range(slots):
                accs = slot_accs[s]
                if not accs:
                    continue
                psf = psfp.tile([BW, CH], F32, tag="psf")
                for j, accT in enumerate(accs):
                    nc.tensor.matmul(
                        out=psf[:], lhsT=accT[:], rhs=wt_sb[:],
                        start=(j == 0), stop=(j == len(accs) - 1),
                    )
                res = resp.tile([BW, CH], F32, tag="res")
                nc.scalar.activation(
                    out=res[:], in_=psf[:],
                    func=mybir.ActivationFunctionType.Relu
                )
                nc.sync.dma_start(out=out_d[s * BW:(s + 1) * BW, :], in_=res[:])


_NC_CACHE = {}


def prepare(x, W, src, dst, vals, n_cores=N_CORES, h_split=H_SPLIT):
    """Host-side planning + input maps."""
    x = np.asarray(x, dtype=np.float32)
    W = np.asarray(W, dtype=np.float32)
    src = np.asarray(src).astype(np.int64)
    dst = np.asarray(dst).astype(np.int64)
    vals = np.asarray(vals, dtype=np.float32)

    n = x.shape[0]
    plan, idxL, idxH, mdl, mvl, msrc = _plan_edges(
        src, dst, vals, n, n_cores, h_split)

    xrows = math.ceil(n / XPAD) * XPAD
    x_pad = np.zeros((xrows, CH), dtype=ml_dtypes.bfloat16)
    x_pad[:n] = x.astype(ml_dtypes.bfloat16)
    wt = np.ascontiguousarray(W.T).astype(ml_dtypes.bfloat16)

    key = (xrows, n_cores, plan["h"], plan["KL"], plan["KH"], IDG,
           tuple(plan["TL"]), tuple(plan["TH"]))
    nc = _NC_CACHE.get(key)
    if nc is None:
        nc = _build_nc(xrows, plan, n_cores)
        _NC_CACHE[key] = nc

    in_maps = []
    for c in range(n_cores):
        m = {"xtab": x_pad, "wt": wt, "mdl": mdl[c], "mvl": mvl[c]}
        if IDG:
            m["msrc"] = msrc[c]
        if plan["KL"] and not IDG:
            m["idxlo"] = idxL[c]
        if plan["KH"] and not IDG:
            m["idxhi"] = idxH[c]
        in_maps.append(m)
    return nc, in_maps, plan, n


def assemble(results, plan, n, n_cores=N_CORES):
    """Scatter per-core slot outputs back to the full [n, CH] output."""
    assign, slots = plan["assign"], plan["slots"]
    out_full = np.zeros((slots * n_cores * BW, CH), dtype=np.float32)
    for c in range(n_cores):
        o = results[c]["out"]
        for s in range(slots):
            b = int(assign[s, c])
            out_full[b * BW:(b + 1) * BW] = o[s * BW:(s + 1) * BW]
    return out_full[:n]


def kernel(x, W, src, dst, vals, **_run_kwargs):
    nc, in_maps, plan, n = prepare(x, W, src, dst, vals)
    res = run_bass_kernel_spmd(
        nc, in_maps, core_ids=list(range(N_CORES)), **_run_kwargs
    )
    out = assemble(res.results, plan, n)
    if _run_kwargs:
        return out, res
    return out


# revision 28
# speedup vs baseline: 4.1048x; 1.2382x over previous
"""CCConvLayer (GNN message passing) on 8 Trainium2 NeuronCores.

Reference:
    x1  = x @ W.T                      # dense projection [N, 128]
    out = relu(segment_sum(x1[src] * vals[:, None], dst, N))

v2 strategy — the projection commutes with the (linear) segment-sum:
    out = relu(segment_sum(x[src] * vals[:, None], dst, N) @ W.T)
so there is NO phase-1 x1 table at all.  Per core:

  * dst space is cut into 128-node blocks; blocks are assigned to
    (core, slot) pairs balanced by edge count.  Every core owns the full
    output rows of its blocks => no collective; the host re-assembles.
  * Edges gather raw x rows (bf16, 256B) straight from DRAM with
    dma_gather (int16 indices => lo/hi table halves, sorted by src for
    HBM locality).
  * Scatter-add per 128-edge tile: one-hot S[e, n] = vals[e] *
    (dst_local[e] == n); the tile matmul uses the GATHERED tile as
    lhsT:  psum[ci, n] += G[e, ci].T-contraction S[e, n], which
    accumulates the TRANSPOSED block sum, so the final W application is
    a single plain matmul per slot:
        out[n, co] = relu( accT[ci, n].T @ W.T[ci, co] )  (lhsT=accT!)
  * S tiles are built on TWO engines to halve the critical path:
    DVE scalar_tensor_tensor (is_equal * val) and Pool local_scatter
    (8-tile batches, idx = dst_local + 128*batch_pos, -1 pads skipped).
  * psum->SBUF copies and the final ReLU run on the Activation engine,
    which is otherwise idle.
"""

import math

import numpy as np
import ml_dtypes

import concourse.bacc as bacc
import concourse.bass as bass
import concourse.mybir as mybir
import concourse.tile as tile
from concourse.bass_utils import run_bass_kernel_spmd

P = 128          # partitions / edge-tile size
BW = 128         # dst-block width (S-matrix columns, psum free dim)
CH = 128         # in/out channels (problem-specific)
N_CORES = 8
H_SPLIT = 20224  # table half-split: each phase's working set fits the DMA cache
                 # (must stay < 32768 for int16 dma_gather indices)
GC = 16          # gather chunk size in tiles (2048 edges / call)
NQ = 4           # SWDGE queues used for gathers
SP = False       # single_packet for dma_gather
GB = 24          # gather pool buffers
IDG = 0          # 1 = per-tile indirect_dma_start gather (int32 offsets)
IDT = 1          # tiles per indirect_dma_start call
NEGPAD = 0       # 1 = pad gather idxs are -1 (broken: crashes DGE); 2 = memset pad tiles only
SORT = 1         # sort each (slot, section)'s edges by src
XPAD = 512       # x row padding granularity
SB = 16          # edge-tiles per wide-S DVE build (2 ops per batch)
ACCB = 88        # accT pool buffers
WS = 0           # 1 = wide-S batched builds; 0 = per-tile eager stt builds
SELB = 110       # S pool buffers

F32 = mybir.dt.float32
BF16 = mybir.dt.bfloat16
I32 = mybir.dt.int32
I16 = mybir.dt.int16


def _wrap_idx(idx):
    """int16 index layout for dma_gather: element i at partition i%16,
    column i//16; 16-partition block replicated to all 128 partitions."""
    L = len(idx) // 16
    w = idx.reshape(L, 16).T.astype(np.int16)  # [16, L]
    return np.ascontiguousarray(np.tile(w, (8, 1)))  # [128, L]


def _plan_edges(src, dst, vals, n_nodes, n_cores, h_split):
    """Bucket edges by 128-node dst block, assign blocks to (slot, core),
    split each slot's edges into lo (src < h_split) / hi sections, pad each
    (slot, core, section) to T*128 edges shared across cores."""
    nb = math.ceil(n_nodes / BW)
    nb_pad = math.ceil(nb / n_cores) * n_cores
    slots = nb_pad // n_cores

    blk = (dst // BW).astype(np.int64)
    counts = np.bincount(blk, minlength=nb_pad)
    order = np.argsort(-counts, kind="stable")

    assign = np.empty((slots, n_cores), dtype=np.int64)
    totals = np.zeros(n_cores, dtype=np.int64)
    for s in range(slots):
        group = order[s * n_cores:(s + 1) * n_cores]  # desc counts
        cs = np.argsort(totals, kind="stable")  # least-loaded cores first
        for i, b in enumerate(group):
            assign[s, cs[i]] = b
            totals[cs[i]] += counts[b]

    # per-edge-per-core grouping
    eorder = np.argsort(blk, kind="stable")
    starts = np.zeros(nb_pad + 1, dtype=np.int64)
    np.cumsum(counts, out=starts[1:])

    # edge lists per (slot, core, section)
    lists = [[None] * n_cores for _ in range(slots)]
    TL = np.zeros(slots, dtype=np.int64)
    TH = np.zeros(slots, dtype=np.int64)
    for s in range(slots):
        for c in range(n_cores):
            b = int(assign[s, c])
            e = eorder[starts[b]:starts[b + 1]]
            lo = e[src[e] < h_split]
            hi = e[src[e] >= h_split]
            # sort by src: descriptors then walk the x table monotonically,
            # turning random 256B HBM reads into row-local ones
            if SORT:
                lo = lo[np.argsort(src[lo], kind="stable")]
                hi = hi[np.argsort(src[hi], kind="stable")]
            lists[s][c] = (lo, hi)
            TL[s] = max(TL[s], -(-len(lo) // P))
            TH[s] = max(TH[s], -(-len(hi) // P))
        if TL[s] + TH[s] == 0:
            TL[s] = 1  # keep the psum chain non-empty
    KL = int(TL.sum())
    KH = int(TH.sum())
    K = KL + KH
    offL = np.zeros(slots + 1, dtype=np.int64)
    np.cumsum(TL, out=offL[1:])
    offH = np.zeros(slots + 1, dtype=np.int64)
    np.cumsum(TH, out=offH[1:])

    # global tile order: per slot, lo tiles then hi tiles (matches emission)
    # guv[u] = global order position of meta column u (u = lo: offL[s]+t,
    # hi: KL + offH[s]+t)
    guv = np.zeros(K, dtype=np.int64)
    g = 0
    for s in range(slots):
        for t in range(TL[s]):
            guv[offL[s] + t] = g
            g += 1
    for s in range(slots):
        for t in range(TH[s]):
            guv[KL + offH[s] + t] = g
            g += 1
    fill = -1 if NEGPAD == 1 else 0
    srcL = np.full((n_cores, KL * P), fill, dtype=np.int64)
    srcH = np.full((n_cores, KH * P), fill, dtype=np.int64)
    dstl_a = np.zeros((n_cores, K * P), dtype=np.float32)
    val_a = np.zeros((n_cores, K * P), dtype=np.float32)
    srca_a = np.zeros((n_cores, K * P), dtype=np.int64)
    for s in range(slots):
        for c in range(n_cores):
            b = int(assign[s, c])
            lo, hi = lists[s][c]
            ll = int(offL[s]) * P
            srcL[c, ll:ll + len(lo)] = src[lo]
            srca_a[c, ll:ll + len(lo)] = src[lo]
            dstl_a[c, ll:ll + len(lo)] = (dst[lo] - b * BW).astype(np.float32)
            val_a[c, ll:ll + len(lo)] = vals[lo]
            ho = int(offH[s]) * P
            srcH[c, ho:ho + len(hi)] = src[hi] - h_split
            hh = (KL + int(offH[s])) * P
            srca_a[c, hh:hh + len(hi)] = src[hi]
            dstl_a[c, hh:hh + len(hi)] = (dst[hi] - b * BW).astype(np.float32)
            val_a[c, hh:hh + len(hi)] = vals[hi]

    idxL = np.stack([_wrap_idx(srcL[c]) for c in range(n_cores)]) \
        if KL else np.zeros((n_cores, P, 0), dtype=np.int16)
    idxH = np.stack([_wrap_idx(srcH[c]) for c in range(n_cores)]) \
        if KH else np.zeros((n_cores, P, 0), dtype=np.int16)

    # interleave dstl/vals: position j -> (tile j//P, partition j%P) => [P, K]
    dstl_i = dstl_a.reshape(n_cores, K, P).transpose(0, 2, 1)  # [C, P, K]
    val_i = val_a.reshape(n_cores, K, P).transpose(0, 2, 1)
    srca_i = srca_a.reshape(n_cores, K, P).transpose(0, 2, 1)
    # re-index S metadata columns into GLOBAL EMISSION order (per slot:
    # lo tiles then hi tiles), so pool batches slice contiguous columns
    inv = np.argsort(guv)  # inv[g] = meta col u at global position g
    dstl_g = dstl_i[:, :, inv]
    val_g = val_i[:, :, inv]
    msrc = np.ascontiguousarray(srca_i[:, :, inv]).astype(np.int32)

    if WS:
        # dl2[p, g] = dst_local + BW * (g % SB): column index within the
        # wide-S batch buffer [P, SB*BW].  f32 (bf16 cannot represent
        # 0..SB*BW-1 exactly)
        off = (np.arange(K) % SB) * BW
        mdl = np.ascontiguousarray(
            (dstl_g + off[None, None, :]).astype(np.float32))
    else:
        # per-tile stt: plain dst_local (< BW <= 128, bf16-exact)
        mdl = np.ascontiguousarray(dstl_g.astype(ml_dtypes.bfloat16))
    mvl = np.ascontiguousarray(val_g.astype(ml_dtypes.bfloat16))

    # tiles (sec, t) that contain pad slots in ANY core (memset before gather)
    padset = set()
    for s_ in range(slots):
        for c in range(n_cores):
            lo, hi = lists[s_][c]
            for ln, T_, off_, sec in ((len(lo), TL[s_], offL[s_], 0),
                                      (len(hi), TH[s_], offH[s_], 1)):
                full = -(-ln // P) if ln else 0
                for t_ in range(full - 1 if ln % P else full, T_):
                    if t_ >= 0:
                        padset.add((sec, int(off_) + t_))
    plan = {
        "assign": assign,
        "h": h_split,
        "slots": slots,
        "TL": TL.tolist(),
        "TH": TH.tolist(),
        "KL": KL,
        "KH": KH,
        "offL": offL.tolist(),
        "offH": offH.tolist(),
        "padset": sorted(padset),
    }
    return plan, idxL, idxH, mdl, mvl, msrc


def _build_nc(xrows, plan, n_cores, loop_n=1, mode="full"):
    """Build the SPMD Bass program (identical on every core).

    loop_n > 1 wraps the body in an on-device repeat loop; mode
    ("full" | "gonly" | "nog" | "nos") ablates phases for timing."""
    nc = bacc.Bacc(
        "TRN2",
        target_bir_lowering=False,
        debug=False,
        enable_asserts=False,
        num_devices=n_cores,
        num_swdge_queues=NQ,
    )
    KL, KH = plan["KL"], plan["KH"]
    K = KL + KH
    slots = plan["slots"]
    # raw x rows, node-major bf16 [xrows, CH] — the gather table
    xt_d = nc.dram_tensor("xtab", [xrows, CH], BF16, kind="ExternalInput").ap()
    wt_d = nc.dram_tensor("wt", [CH, CH], BF16, kind="ExternalInput").ap()
    mf_d = nc.dram_tensor("mdl", [P, K], F32 if WS else BF16,
                          kind="ExternalInput").ap()
    mi_d = nc.dram_tensor("mvl", [P, K], BF16, kind="ExternalInput").ap()
    ms_d = (nc.dram_tensor("msrc", [P, K], I32, kind="ExternalInput").ap()
            if IDG else None)
    il_d = (
        nc.dram_tensor("idxlo", [P, KL * 8], I16, kind="ExternalInput").ap()
        if KL and not IDG else None
    )
    ih_d = (
        nc.dram_tensor("idxhi", [P, KH * 8], I16, kind="ExternalInput").ap()
        if KH and not IDG else None
    )
    out_d = nc.dram_tensor("out", [slots * BW, CH], F32, kind="ExternalOutput").ap()

    with tile.TileContext(nc) as tc:
        if loop_n > 1:
            with tc.For_i(0, loop_n, 1):
                _emit_body(nc, tc, plan, xrows, xt_d, wt_d, mf_d, mi_d,
                           il_d, ih_d, ms_d, out_d, mode)
        else:
            _emit_body(nc, tc, plan, xrows, xt_d, wt_d, mf_d, mi_d,
                       il_d, ih_d, ms_d, out_d, mode)
    nc.compile()
    return nc


def _emit_body(nc, tc, plan, xrows, xt_d, wt_d, mf_d, mi_d, il_d, ih_d,
               ms_d, out_d, mode="full"):
    slots = plan["slots"]
    TL, TH = plan["TL"], plan["TH"]
    KL, KH = plan["KL"], plan["KH"]
    offL, offH = plan["offL"], plan["offH"]
    K = KL + KH

    with (
        tc.tile_pool(name="const", bufs=1) as constp,
        tc.tile_pool(name="gat", bufs=GB) as gp,
        tc.tile_pool(name="sel", bufs=SELB) as selp,
        tc.tile_pool(name="acc", bufs=ACCB) as accp,
        tc.tile_pool(name="res", bufs=3) as resp,
        tc.tile_pool(name="ps", bufs=6, space="PSUM") as psp,
        tc.tile_pool(name="psf", bufs=2, space="PSUM") as psfp,
    ):
        wt_sb = constp.tile([CH, CH], BF16)
        nc.sync.dma_start(out=wt_sb[:], in_=wt_d[:])
        dl_sb = constp.tile([P, K], F32 if WS else BF16)
        nc.sync.dma_start(out=dl_sb[:], in_=mf_d[:])
        vl_sb = constp.tile([P, K], BF16)
        nc.sync.dma_start(out=vl_sb[:], in_=mi_d[:])
        if IDG:
            ms_sb = constp.tile([P, K], I32)
            nc.sync.dma_start(out=ms_sb[:], in_=ms_d[:])
        if KL and not IDG:
            il_sb = constp.tile([P, KL * 8], I16)
            nc.sync.dma_start(out=il_sb[:], in_=il_d[:])
        if KH and not IDG:
            ih_sb = constp.tile([P, KH * 8], I16)
            nc.sync.dma_start(out=ih_sb[:], in_=ih_d[:])
        iw = SB * BW if WS else BW
        iota_i = constp.tile([P, iw], I32)
        nc.gpsimd.iota(iota_i[:], pattern=[[1, iw]], base=0,
                       channel_multiplier=0)
        iota_f = constp.tile([P, iw], F32 if WS else BF16)
        nc.vector.tensor_copy(iota_f[:], iota_i[:])

        # ---- bulk gather: chunked dma_gather per section, lazy issue ----
        chunks = {}  # (sec, chunk_id) -> (tile, tiles_in_chunk)
        qrr = [0]
        padset = set(map(tuple, plan.get("padset", ()))) if NEGPAD else set()

        idg_tiles = {}

        def idg_gather(gi):
            b0 = (gi // IDT) * IDT
            if b0 not in idg_tiles:
                nt = min(IDT, K - b0)
                g = gp.tile([P, nt * CH], BF16, tag="gat")
                nc.gpsimd.indirect_dma_start(
                    out=g[:].rearrange("p (t c) -> p t c", c=CH),
                    out_offset=None,
                    in_=xt_d[:],
                    in_offset=bass.IndirectOffsetOnAxis(
                        ap=ms_sb[:, b0:b0 + nt], axis=0),
                )
                if mode == "gonly":
                    dummy = selp.tile([P, 1], F32, tag="dmy")
                    nc.vector.tensor_copy(dummy[:], g[:, :1])
                idg_tiles[b0] = g
            return idg_tiles[b0][:, ((gi - b0) * CH):((gi - b0 + 1) * CH)]

        def chunk_of(sec, t):
            cid = t // GC
            key = (sec, cid)
            if key not in chunks:
                ksec = KL if sec == 0 else KH
                nt = min(GC, ksec - cid * GC)
                g = gp.tile([P, nt * CH], BF16, tag="gat")
                for tt in range(cid * GC, cid * GC + nt):
                    if (sec, tt) in padset:
                        j = tt - cid * GC
                        nc.any.memset(g[:, j * CH:(j + 1) * CH], 0.0)
                isb = il_sb if sec == 0 else ih_sb
                h = min(plan["h"], xrows)
                table = xt_d[:h, :] if sec == 0 else xt_d[h:, :]
                nc.gpsimd.dma_gather(
                    out_ap=g[:].rearrange("p (t c) -> p t c", c=CH),
                    in_ap=table,
                    idxs_ap=isb[:, cid * GC * 8:(cid * GC + nt) * 8],
                    num_idxs=nt * P,
                    num_idxs_reg=nt * P,
                    elem_size=CH,
                    single_packet=SP,
                    queue_num=qrr[0],
                )
                qrr[0] = (qrr[0] + 1) % NQ
                if not WS:
                    for tt in range(cid * GC, cid * GC + nt):
                        gtt = t2g.get((sec, tt))
                        if gtt is not None:
                            build_s(gtt)

                if mode == "gonly":
                    dummy = selp.tile([P, 1], F32, tag="dmy")
                    nc.vector.tensor_copy(dummy[:], g[:, :1])
                chunks[key] = (g, nt)
            return chunks[key]

        # ---- wide-S: one [P, SB*BW] buffer = SB edge-tiles, 2 DVE ops ----
        wdict = {}
        sdict = {}

        def build_s(gi):
            if gi in sdict or mode in ("nos", "gonly"):
                return
            S = selp.tile([P, BW], BF16, tag="sel")
            nc.vector.scalar_tensor_tensor(
                out=S[:],
                in0=iota_f[:, :BW],
                scalar=dl_sb[:, gi:gi + 1],
                in1=vl_sb[:, gi:gi + 1].to_broadcast([P, BW]),
                op0=mybir.AluOpType.is_equal,
                op1=mybir.AluOpType.mult,
            )
            sdict[gi] = S

        def wide_s(b):
            if b not in wdict:
                nt = min(SB, K - b * SB)
                W_ = selp.tile([P, SB * BW], BF16, tag="selw")
                c0 = b * SB
                nc.vector.tensor_tensor(
                    out=W_[:, :nt * BW],
                    in0=dl_sb[:, c0:c0 + nt].to_broadcast([P, nt, BW]),
                    in1=iota_f[:, :nt * BW].rearrange(
                        "p (t n) -> p t n", n=BW),
                    op=mybir.AluOpType.is_equal,
                )
                nc.vector.tensor_tensor(
                    out=W_[:, :nt * BW],
                    in0=W_[:, :nt * BW],
                    in1=vl_sb[:, c0:c0 + nt].to_broadcast([P, nt, BW]),
                    op=mybir.AluOpType.mult,
                )
                wdict[b] = W_
            return wdict[b]

        # ---- two-phase psum chains: all lo sections, then all hi ----
        acc_lo = {}
        g = 0  # global tile position (emission order)
        t2g = {}
        _gg = 0
        for _s in range(slots):
            for _t in range(TL[_s]):
                t2g[(0, offL[_s] + _t)] = _gg
                _gg += 1
        for _s in range(slots):
            for _t in range(TH[_s]):
                t2g[(1, offH[_s] + _t)] = _gg
                _gg += 1

        def chain(sec, s, T, off):
            nonlocal g
            tiles = [(sec, off + t) for t in range(T)]
            if mode == "gonly":
                for i, (sc, t) in enumerate(tiles):
                    idg_gather(g + i) if IDG else chunk_of(sc, t)
                g += len(tiles)
                return None
            ps = psp.tile([CH, BW], F32, tag="ps")
            for i, (sc, t) in enumerate(tiles):
                gi = g + i  # global tile position == S-meta column
                if mode == "nog":
                    gt = wt_sb[:]
                elif IDG:
                    gt = idg_gather(gi)
                else:
                    gtile, _ = chunk_of(sc, t)
                    gt = gtile[:, (t % GC) * CH:(t % GC + 1) * CH]
                if mode == "nos":
                    S_ap = wt_sb[:, :BW]
                elif WS:
                    W_ = wide_s(gi // SB)
                    j = gi % SB
                    S_ap = W_[:, j * BW:(j + 1) * BW]
                else:
                    if gi not in sdict:
                        build_s(gi)
                    S_ap = sdict.pop(gi)[:]
                nc.tensor.matmul(
                    out=ps[:],
                    lhsT=gt,
                    rhs=S_ap,
                    start=(i == 0),
                    stop=(i == len(tiles) - 1),
                )
            g += len(tiles)
            accT = accp.tile([CH, BW], BF16, tag="acc")
            nc.scalar.copy(accT[:], ps[:])
            return accT

        slot_accs = {s: [] for s in range(slots)}
        for s in range(slots):
            if TL[s]:
                a = chain(0, s, TL[s], offL[s])
                if a is not None:
                    slot_accs[s].append(a)
        for s in range(slots):
            if TH[s]:
                a = chain(1, s, TH[s], offH[s])
                if a is not None:
                    slot_accs[s].append(a)
        # final W application deferred so PE never stalls mid-stream on the
        # chain-end copies
        if mode != "gonly":
            for s in range(slots):
                accs = slot_accs[s]
                if not accs:
                    continue
                psf = psfp.tile([BW, CH], F32, tag="psf")
                for j, accT in enumerate(accs):
                    nc.tensor.matmul(
                        out=psf[:], lhsT=accT[:], rhs=wt_sb[:],
                        start=(j == 0), stop=(j == len(accs) - 1),
                    )
                res = resp.tile([BW, CH], F32, tag="res")
                nc.scalar.activation(
                    out=res[:], in_=psf[:],
                    func=mybir.ActivationFunctionType.Relu
                )
                nc.sync.dma_start(out=out_d[s * BW:(s + 1) * BW, :], in_=res[:])


_NC_CACHE = {}


def prepare(x, W, src, dst, vals, n_cores=N_CORES, h_split=None):
    """Host-side planning + input maps."""
    if h_split is None:
        h_split = H_SPLIT
    x = np.asarray(x, dtype=np.float32)
    W = np.asarray(W, dtype=np.float32)
    src = np.asarray(src).astype(np.int64)
    dst = np.asarray(dst).astype(np.int64)
    vals = np.asarray(vals, dtype=np.float32)

    n = x.shape[0]
    plan, idxL, idxH, mdl, mvl, msrc = _plan_edges(
        src, dst, vals, n, n_cores, h_split)

    xrows = math.ceil(n / XPAD) * XPAD
    x_pad = np.zeros((xrows, CH), dtype=ml_dtypes.bfloat16)
    x_pad[:n] = x.astype(ml_dtypes.bfloat16)
    wt = np.ascontiguousarray(W.T).astype(ml_dtypes.bfloat16)

    key = (xrows, n_cores, plan["h"], plan["KL"], plan["KH"], IDG,
           tuple(plan["TL"]), tuple(plan["TH"]))
    nc = _NC_CACHE.get(key)
    if nc is None:
        nc = _build_nc(xrows, plan, n_cores)
        _NC_CACHE[key] = nc

    in_maps = []
    for c in range(n_cores):
        m = {"xtab": x_pad, "wt": wt, "mdl": mdl[c], "mvl": mvl[c]}
        if IDG:
            m["msrc"] = msrc[c]
        if plan["KL"] and not IDG:
            m["idxlo"] = idxL[c]
        if plan["KH"] and not IDG:
            m["idxhi"] = idxH[c]
        in_maps.append(m)
    return nc, in_maps, plan, n


def assemble(results, plan, n, n_cores=N_CORES):
    """Scatter per-core slot outputs back to the full [n, CH] output."""
    assign, slots = plan["assign"], plan["slots"]
    out_full = np.zeros((slots * n_cores * BW, CH), dtype=np.float32)
    for c in range(n_cores):
        o = results[c]["out"]
        for s in range(slots):
            b = int(assign[s, c])
            out_full[b * BW:(b + 1) * BW] = o[s * BW:(s + 1) * BW]
    return out_full[:n]


def kernel(x, W, src, dst, vals, **_run_kwargs):
    nc, in_maps, plan, n = prepare(x, W, src, dst, vals)
    res = run_bass_kernel_spmd(
        nc, in_maps, core_ids=list(range(N_CORES)), **_run_kwargs
    )
    out = assemble(res.results, plan, n)
    if _run_kwargs:
        return out, res
    return out
